# revision 17
# baseline (speedup 1.0000x reference)
"""Trainium2 kernel for per-task MLP routing (MoE-style dictionary model).

Computation (reference):
    l1 = l1_emb[task_ids] -> [B, 256, 64]; l2 = l2_emb[task_ids] -> [B, 64, 64]
    l3 = l3_emb[task_ids] -> [B, 64]
    h1 = gelu(x @ l1); h2 = gelu(h1 @ l2); out = sigmoid(sum(h2*l3))  [B, 1]

Strategy: expert-parallel over tasks. Tasks t in [128*c, 128*(c+1)) live on
core c. The host routes samples to cores by task id, groups each task's
samples into fixed-capacity slots (CAP rows), and pre-gathers the per-slot
weights. Two slots are fused per "pair": their W1 k-half blocks sit side by
side in one 128x128 stationary, and their W2s form one 128x128 block-diagonal
stationary, so every LDWEIGHTS is a full 128-column load (triggers the
compiler's fast-weight-load, 32b/cycle reads) over fp8 weights (4 elem/read,
and half the HBM traffic of bf16). Activations stay bf16 (mixed-dtype matmul).
Both slots' samples stream against the fused stationary; the off-diagonal
half of each PSUM result is garbage that the host simply never reads.

Per group of 16 pairs (one full 512-col PSUM bank): 32 L1 matmuls (k-split
accumulation), Gelu (ACT, scale 1/16 undoes the fp8 weight scaling), 16 L2
matmuls, Gelu, then one L3 matmul with the per-slot l3 vectors as a [128, 32]
stationary (row e*64+h of col s holds l3[slot s]) producing per-sample logits
on 32 PSUM rows. Sigmoid is computed as 0.5*tanh(logit/2)+0.5 -- Tanh lives
in the same ACT table set as Gelu (no ~2.7us table swap), and the affine runs
on the otherwise-idle Vector engine.
"""

import numpy as np

F = 256          # features
H = 64           # hidden
NT = 1024        # num tasks
NCORES = 8
TPC = NT // NCORES   # tasks per core
CAP = 16             # sample rows per slot
GP = 16              # pairs per group (16 pairs * 32 cols = 512 psum cols)
WSCALE = 16.0        # fp8 weight pre-scale (undone inside the Gelu ACT)
USE_DR = False       # fuse L1's two k-half matmuls via fp8 DoubleRow (contraction 256)
PIPE_SKEW = True     # software-pipeline stages across groups (L2 lags L1 by 1, L3 by 2)

_PROGRAM_CACHE = {}
LAST_IN_MAPS = None  # stashed for test.py's timing harness
LAST_NPAIRS = None


ALL_PARTS = frozenset({"dma", "l1", "act1", "l2", "act2", "l3", "sig", "out"})
# "touch": tiny DVE read of each DMA'd weight tile, forcing DMA completion to
# be observable in DMA-only ablation variants.


def _build_program(n_pairs, passes=1, parts=None, loop_iters=None):
    from contextlib import ExitStack

    import concourse.bacc as bacc
    import concourse.tile as tile
    from concourse import mybir

    if parts is None:
        parts = ALL_PARTS

    f32 = mybir.dt.float32
    bf16 = mybir.dt.bfloat16
    f8 = mybir.dt.float8e4
    S = 2 * n_pairs
    NG = (n_pairs + GP - 1) // GP

    nc = bacc.Bacc("TRN2", target_bir_lowering=False)
    xs_d = nc.declare_dram_parameter("xs", [128, 2 * S * CAP], f8, False)
    wg_d = nc.declare_dram_parameter("wg", [128, n_pairs * 384], f8, False)
    l3_d = nc.declare_dram_parameter("l3p", [128, S], bf16, False)
    f16 = mybir.dt.float16
    out_d = nc.declare_dram_parameter("out", [NG, 32, 512], f16, True)

    GELU = mybir.ActivationFunctionType.Gelu
    TANH = mybir.ActivationFunctionType.Tanh

    with ExitStack() as ctx:
        tc = ctx.enter_context(tile.TileContext(nc))
        singles = ctx.enter_context(tc.tile_pool(name="singles", bufs=1))
        wpool = ctx.enter_context(tc.tile_pool(name="wpool", bufs=4))
        hpool = ctx.enter_context(tc.tile_pool(name="hpool", bufs=2))
        opool = ctx.enter_context(tc.tile_pool(name="opool", bufs=2))
        p1pool = ctx.enter_context(tc.tile_pool(name="psum1", bufs=2, space="PSUM"))
        p2pool = ctx.enter_context(tc.tile_pool(name="psum2", bufs=2, space="PSUM"))
        p3pool = ctx.enter_context(tc.tile_pool(name="psum3", bufs=2, space="PSUM"))

        xs_sb = singles.tile([128, 2 * S * CAP], f8, tag="xs")
        nc.sync.dma_start(out=xs_sb, in_=xs_d[:])
        xs3 = xs_sb.rearrange("p (k c) -> p k c", k=2)
        l3_sb = singles.tile([128, S], bf16, tag="l3p")
        nc.sync.dma_start(out=l3_sb, in_=l3_d[:])

        # Software-pipelined group loop with stage skew: iteration i emits
        # L1(seq[i]), L2(seq[i-1]), L3(seq[i-2]) so the PE never head-of-line
        # blocks on an ACT result -- gelu1(g-1) completes while L1(g) streams.
        seq = [g % NG for g in range(NG * passes)]
        n = len(seq)

        from contextlib import nullcontext
        loop_cm = tc.For_i(0, loop_iters) if loop_iters else nullcontext()
        ctx.enter_context(loop_cm)

        def ginfo(g):
            p0 = g * GP
            GPg = min(GP, n_pairs - p0)
            return p0, GPg, GPg * 32, 2 * GPg

        st1 = {}  # g-slot -> (ps1, wgt, GPg)
        st2 = {}  # g-slot -> (ps2, wgt, h1, GPg)
        st3 = {}  # g-slot -> (ps3, GPg)

        skew1, skew2 = (1, 2) if PIPE_SKEW else (0, 0)
        for i in range(n + 2 if PIPE_SKEW else n):
            if i < n:
                g = seq[i]
                p0, GPg, GC, NS = ginfo(g)
                wgt = wpool.tile([128, GPg * 384], f8, tag="wg")
                if "dma" in parts:
                    nc.sync.dma_start(
                        out=wgt, in_=wg_d[:, p0 * 384 : p0 * 384 + GPg * 384]
                    )
                if "touch" in parts:
                    dumt_full = opool.tile([1, 2], f32, tag="dum")
                    nc.vector.tensor_copy(dumt_full, wgt[0:1, 0:2])

                # Layer 1: both slots of a pair share one [128,128] stationary;
                # psum rows 0:64 = even slot's h, rows 64:128 = odd slot's.
                ps1_full = p1pool.tile([128, 512], f32, tag="ps1")
                ps1 = ps1_full[:, :GC]
                wgt4 = wgt.rearrange("p (k q m) -> p k q m", k=3, q=GPg)
                for pr in range(GPg) if "l1" in parts else ():
                    pc = slice(pr * 32, (pr + 1) * 32)
                    if USE_DR:
                        nc.tensor.matmul(
                            out=ps1[:, pc],
                            lhsT=wgt4[:, 0:2, pr, :],
                            rhs=xs3[:, :, (p0 + pr) * 32 : (p0 + pr + 1) * 32],
                            start=True,
                            stop=True,
                            perf_mode=mybir.MatmulPerfMode.DoubleRow,
                        )
                    else:
                        for k in range(2):
                            nc.tensor.matmul(
                                out=ps1[:, pc],
                                lhsT=wgt[:, (k * GPg + pr) * 128 : (k * GPg + pr + 1) * 128],
                                rhs=xs3[:, k, (p0 + pr) * 32 : (p0 + pr + 1) * 32],
                                start=(k == 0),
                                stop=(k == 1),
                            )
                h1_full = hpool.tile([128, 512], bf16, tag="h1")
                h1 = h1_full[:, :GC]
                if "act1" in parts:
                    nc.scalar.activation(out=h1, in_=ps1, func=GELU, scale=1.0 / WSCALE)
                st1[i] = (h1, wgt, g)

            if i - skew1 >= 0 and i - skew1 < n:
                h1, wgt, g = st1.pop(i - skew1)
                p0, GPg, GC, NS = ginfo(g)
                # Layer 2: block-diagonal [128,128] stationary per pair.
                ps2_full = p2pool.tile([128, 512], f32, tag="ps2")
                ps2 = ps2_full[:, :GC]
                for pr in range(GPg) if "l2" in parts else ():
                    pc = slice(pr * 32, (pr + 1) * 32)
                    nc.tensor.matmul(
                        out=ps2[:, pc],
                        lhsT=wgt[:, (2 * GPg + pr) * 128 : (2 * GPg + pr + 1) * 128],
                        rhs=h1[:, pc],
                        start=True,
                        stop=True,
                    )
                h2_full = hpool.tile([128, 512], bf16, tag="h2")
                h2 = h2_full[:, :GC]
                if "act2" in parts:
                    nc.scalar.activation(out=h2, in_=ps2, func=GELU, scale=1.0 / WSCALE)
                st2[i - skew1] = (h2, g)

            if i - skew2 >= 0 and i - skew2 < n:
                h2, g = st2.pop(i - skew2)
                p0, GPg, GC, NS = ginfo(g)
                # Layer 3: per-slot l3 columns as stationary; psum row = local
                # slot index, valid at that slot's sample columns.
                ps3_full = p3pool.tile([128, 512], f32, tag="ps3")
                ps3 = ps3_full[:NS, :GC]
                if "l3" in parts:
                    nc.tensor.matmul(
                        out=ps3,
                        lhsT=l3_sb[:, 32 * g : 32 * g + NS],
                        rhs=h2,
                        start=True,
                        stop=True,
                    )
                # sigmoid(x) = 0.5*tanh(x/2) + 0.5; Tanh shares Gelu's ACT
                # table set, the affine runs on the idle Vector engine.
                tt_full = hpool.tile([32, 512], bf16, tag="t")
                tt = tt_full[:NS, :GC]
                outt_full = opool.tile([32, 512], f16, tag="o")
                outt = outt_full[:NS, :GC]
                if "sig" in parts:
                    nc.scalar.activation(out=tt, in_=ps3, func=TANH, scale=0.5)
                    nc.vector.tensor_scalar(
                        outt, tt, 0.5, 0.5, mybir.AluOpType.mult, mybir.AluOpType.add
                    )
                if "out" in parts:
                    nc.sync.dma_start(out=out_d[g, :NS, :GC], in_=outt)

    nc.compile()
    return nc


def _route(tids):
    """Group sample indices by task, pack into CAP-row slots per core.

    Returns (n_pairs, slot_task [NCORES, S], slot_sample [NCORES, S, CAP]).
    slot_sample is -1 where padded; slot_task is 0 for unused slots.
    """
    order = np.argsort(tids, kind="stable")
    counts = np.bincount(tids, minlength=NT)
    starts = np.zeros(NT + 1, dtype=np.int64)
    np.cumsum(counts, out=starts[1:])

    per_core = []
    for c in range(NCORES):
        slots = []  # (task, start_in_order, n)
        for t in range(c * TPC, (c + 1) * TPC):
            ct = int(counts[t])
            off = int(starts[t])
            while ct > 0:
                n = min(ct, CAP)
                slots.append((t, off, n))
                off += n
                ct -= n
        per_core.append(slots)

    s_needed = max(len(s) for s in per_core)
    S = max(4, ((s_needed + 1) // 2) * 2)
    n_pairs = S // 2

    slot_task = np.zeros((NCORES, S), dtype=np.int64)
    slot_sample = np.full((NCORES, S, CAP), -1, dtype=np.int64)
    for c in range(NCORES):
        for i, (t, off, n) in enumerate(per_core[c]):
            slot_task[c, i] = t
            slot_sample[c, i, :n] = order[off : off + n]
    return n_pairs, slot_task, slot_sample


def kernel(x, task_ids, l1_emb, l2_emb, l3_emb):
    import ml_dtypes
    from concourse import mybir

    f8_np = mybir.dt.np(mybir.dt.float8e4)
    bf16_np = ml_dtypes.bfloat16

    x = np.asarray(x, dtype=np.float32)
    tids = np.asarray(task_ids).astype(np.int64)
    l1 = np.asarray(l1_emb, dtype=np.float32) * WSCALE
    l2 = np.asarray(l2_emb, dtype=np.float32) * WSCALE
    l3 = np.asarray(l3_emb, dtype=np.float32)
    B = x.shape[0]

    n_pairs, slot_task, slot_sample = _route(tids)
    S = 2 * n_pairs
    NG = (n_pairs + GP - 1) // GP

    in_maps = []
    for c in range(NCORES):
        st = slot_task[c]
        ss = slot_sample[c]
        valid = ss >= 0

        # xs[k, p, s*CAP+j] = x[sample(s,j), 128*k+p]  (0 when padded)
        xg = x[np.where(valid, ss, 0).ravel()]
        xg[~valid.ravel()] = 0.0
        xs2 = np.ascontiguousarray(xg.T.reshape(2, 128, S * CAP)).astype(f8_np)
        xs = np.ascontiguousarray(np.concatenate([xs2[0], xs2[1]], axis=1))

        # w1[k][p, pr*128 + e*64 + h] = WSCALE*W1[slot 2pr+e][128k+p, h]
        w1_all = l1[st].reshape(S, F, H)
        w1 = np.ascontiguousarray(
            w1_all.reshape(n_pairs, 2, F, H).transpose(2, 0, 1, 3).reshape(F, n_pairs * 128)
        ).reshape(2, 128, n_pairs * 128)

        # Block-diagonal W2: bd[e*64+i, pr*128+e*64+j] = WSCALE*W2[slot 2pr+e][i,j]
        w2_all = l2[st].reshape(n_pairs, 2, H, H)
        bd = np.zeros((n_pairs, 2, H, 2, H), dtype=np.float32)
        bd[:, 0, :, 0, :] = w2_all[:, 0]
        bd[:, 1, :, 1, :] = w2_all[:, 1]
        w2bd = np.ascontiguousarray(
            bd.transpose(1, 2, 0, 3, 4).reshape(128, n_pairs * 128)
        )

        # Fused per-group weight buffer: [w1k0 | w1k1 | w2bd] per group.
        wg = np.empty((128, n_pairs * 384), dtype=f8_np)
        for g in range(NG):
            p0 = g * GP
            GPg = min(GP, n_pairs - p0)
            base = p0 * 384
            w = GPg * 128
            wg[:, base : base + w] = w1[0][:, p0 * 128 : p0 * 128 + w].astype(f8_np)
            wg[:, base + w : base + 2 * w] = w1[1][:, p0 * 128 : p0 * 128 + w].astype(f8_np)
            wg[:, base + 2 * w : base + 3 * w] = w2bd[:, p0 * 128 : p0 * 128 + w].astype(f8_np)

        # l3p[e*64+h, s] = l3[slot s][h] for e = s%2, else 0.
        l3_all = l3[st]  # [S, H]
        l3p = np.zeros((2, H, S), dtype=np.float32)
        sidx = np.arange(S)
        l3p[sidx % 2, :, sidx] = l3_all
        l3p = l3p.reshape(128, S).astype(bf16_np)

        in_maps.append({"xs": xs, "wg": wg, "l3p": l3p})

    if n_pairs not in _PROGRAM_CACHE:
        _PROGRAM_CACHE[n_pairs] = _build_program(n_pairs)
    nc = _PROGRAM_CACHE[n_pairs]

    from concourse.bass_utils import run_bass_kernel_spmd

    global LAST_IN_MAPS, LAST_NPAIRS
    LAST_IN_MAPS, LAST_NPAIRS = in_maps, n_pairs
    res = run_bass_kernel_spmd(nc, in_maps, list(range(NCORES)))

    # Unshard: slot s -> group s//32, psum row s%32, col ((s%32)//2)*32+(s%2)*16+j
    y = np.zeros(B, dtype=np.float32)
    sidx = np.arange(S)
    g_idx = sidx // 32
    r_idx = sidx % 32
    cbase = (r_idx // 2) * 32 + (sidx % 2) * 16
    col_idx = cbase[:, None] + np.arange(CAP)[None, :]
    for c in range(NCORES):
        out_c = res.results[c]["out"].astype(np.float32)  # [NG, 32, 512]
        valid = slot_sample[c] >= 0
        gv = np.broadcast_to(g_idx[:, None], (S, CAP))[valid]
        rv = np.broadcast_to(r_idx[:, None], (S, CAP))[valid]
        cv = col_idx[valid]
        y[slot_sample[c][valid]] = out_c[gv, rv, cv]
    return y[:, None]


def measure_hw_ns(in_maps, n_pairs, passes=65, base_passes=17, parts=None):
    """Estimate steady-state HW time per kernel execution.

    Builds a timing variant whose Bass program repeats the full group loop
    `passes` times over the same inputs (one PJRT custom call), and
    differences it against the single-pass program: (T_P - T_1)/(P - 1).
    The multi-ms axon dispatch overhead cancels in the difference.
    """
    import time

    import jax
    from jax.experimental.shard_map import shard_map
    from jax.sharding import Mesh, NamedSharding, PartitionSpec

    import concourse.mybir as mybir
    from concourse.bass2jax import _bass_exec_p, partition_id_tensor

    def runner(nc):
        partition_name = nc.partition_id_tensor.name if nc.partition_id_tensor else None
        in_names, out_names, out_avals = [], [], []
        for alloc in nc.m.functions[0].allocations:
            if not isinstance(alloc, mybir.MemoryLocationSet):
                continue
            name = alloc.memorylocations[0].name
            if alloc.kind == "ExternalInput":
                if name != partition_name:
                    in_names.append(name)
            elif alloc.kind == "ExternalOutput":
                out_names.append(name)
                out_avals.append(
                    jax.core.ShapedArray(
                        tuple(alloc.tensor_shape), mybir.dt.np(alloc.dtype)
                    )
                )
        n_params = len(in_names)
        in_names_all = in_names + out_names + ([partition_name] if partition_name else [])

        def _body(*args):
            operands = list(args)
            if partition_name is not None:
                operands.append(partition_id_tensor())
            return tuple(
                _bass_exec_p.bind(
                    *operands,
                    out_avals=tuple(out_avals),
                    in_names=tuple(in_names_all),
                    out_names=tuple(out_names),
                    lowering_input_output_aliases=(),
                    sim_require_finite=True,
                    sim_require_nnan=True,
                    nc=nc,
                )
            )

        devices = jax.devices()[:NCORES]
        mesh = Mesh(np.asarray(devices), ("core",))
        specs_in = (PartitionSpec("core"),) * (n_params + len(out_names))
        specs_out = (PartitionSpec("core"),) * len(out_names)
        fn = jax.jit(
            shard_map(
                _body, mesh=mesh, in_specs=specs_in, out_specs=specs_out, check_rep=False
            ),
            keep_unused=True,
        )
        sh = NamedSharding(mesh, PartitionSpec("core"))
        args = [
            jax.device_put(
                np.concatenate([np.asarray(m[name]) for m in in_maps], axis=0), sh
            )
            for name in in_names
        ]
        for av in out_avals:
            args.append(
                jax.device_put(
                    np.zeros((NCORES * av.shape[0], *av.shape[1:]), av.dtype), sh
                )
            )
        return fn, args

    pkey = tuple(sorted(parts)) if parts is not None else None
    for p in (base_passes, passes):
        if (n_pairs, p, pkey) not in _PROGRAM_CACHE:
            _PROGRAM_CACHE[(n_pairs, p, pkey)] = _build_program(
                n_pairs, passes=p, parts=parts
            )

    fn1, args1 = runner(_PROGRAM_CACHE[(n_pairs, base_passes, pkey)])
    fnP, argsP = runner(_PROGRAM_CACHE[(n_pairs, passes, pkey)])
    jax.block_until_ready(fn1(*args1))
    jax.block_until_ready(fnP(*argsP))

    def batch(fn, args, k=30):
        t0 = time.perf_counter()
        out = None
        for _ in range(k):
            out = fn(*args)
        jax.block_until_ready(out)
        return time.perf_counter() - t0

    # Pipelined batches amortize the axon dispatch interval; min-of-batches
    # rejects host/neighbor contention windows.
    k = 30
    t1s, tps = [], []
    for _ in range(10):
        t1s.append(batch(fn1, args1, k))
        tps.append(batch(fnP, argsP, k))
    return (min(tps) - min(t1s)) / (k * (passes - base_passes)) * 1e9
